# revision 24
# baseline (speedup 1.0000x reference)
"""Conv2d 3x3 (stride 1, pad 1) as implicit GEMM on 8 Trainium2 NeuronCores.

x: [32, 128, 56, 56] f32, W: [256, 128, 3, 3] f32 -> out: [32, 256, 56, 56] f32

Sharding: data-parallel over batch, 4 images per core.

Split-precision fp8 (e4m3) with DoubleRow matmuls (2 contraction rows/cycle):
  out = (x_hi + x_lo) @ W_hi + x_hi @ W_lo   (+ x_lo @ W_lo on one tap)
with x_hi = e4m3(x), x_lo = e4m3(x - x_hi), W_hi = e4m3(64*W),
W_lo = e4m3(64*W - W_hi); the 64x weight scale keeps W out of the e4m3
subnormal range and is undone in the PSUM->SBUF copy. 27 matmul terms pack
into 14 DoubleRow matmuls per output tile (vs 9 bf16 matmuls = 18 bf16-
equivalent cost), so the PE-bound time drops ~22%. Rel L2 error ~1e-3.

Per-core kernel (PE-bound):
  - host pre-pads x to [4, 128, 2, 58, 58] (hi/lo planes), packs weights as
    [ci, couthalf, 14 pairs, 2, 128] so each DoubleRow matmul's stationary
    operand is one contiguous [128, 2, 128] slice
  - Cin=128 is the contraction dim on the SBUF partition axis; for each
    output tile (img, 8-row group, cout half) 14 DoubleRow matmuls
    accumulate into one PSUM bank; hi/lo planes ride the AP "two" dim for
    same-window pairs, and hand-built overlapping APs pair adjacent taps
  - PSUM -> SBUF copy applies the 1/64 weight unscale (tensor_scalar_mul)
  - input DMAs ride both HWDGE rings, image 0 split into row chunks; a
    dependency-free warmup matmul chain holds the PE clock ramp warm
"""

import sys

for _p in ("/opt/trn_rl_repo",):
    if _p not in sys.path:
        sys.path.insert(0, _p)

import numpy as np
import ml_dtypes

import concourse.bass as bass
import concourse.bacc as bacc
import concourse.mybir as mybir
from concourse import tile
from concourse.ap import AP
from concourse.bass_utils import run_bass_kernel_spmd

N_CORES = 8
B = 32
B_PER_CORE = B // N_CORES  # 4
CIN = 128
COUT = 256
H = W_DIM = 56
HP = WP = 58  # padded
KH = KW = 3
KPOS = KH * KW  # 9
ROWS = 8               # output rows per matmul
NG = H // ROWS         # 7 row groups
NFREE = ROWS * W_DIM   # 448 free dim per matmul (<= 512 psum bank)
COUT_TILES = COUT // 128  # 2
W_SCALE = 64.0

# Tap flat offsets in the padded [58, 58] image: o_k = kh*WP + kw
TAP_OFF = [kh * WP + kw for kh in range(KH) for kw in range(KW)]
# x_hi @ W_lo tap pairs (ka, kb) sharing one DoubleRow matmul; the "two"
# AP dim strides by o_kb - o_ka. Tap 8 is handled as an hl-pair instead.
HI_PAIRS = [(0, 1), (2, 3), (4, 5), (6, 7)]
N_PAIRS = KPOS + len(HI_PAIRS) + 1  # 14

_NC_CACHE = None


def build_nc(reps: int = 1, xsplits=(0, 10, 18, 34, HP), wchunks: int = 2) -> bass.Bass:
    # Bacc (not raw Bass): its compile() legalizes multi-wait instructions
    # for the 1-sync-wait-per-instruction encoding limit of this toolchain.
    nc = bacc.Bacc()
    xp = nc.dram_tensor(
        "xp", [B_PER_CORE, CIN, 2, HP * WP], mybir.dt.float8e4, kind="ExternalInput"
    )
    wt = nc.dram_tensor(
        "wt", [CIN, COUT_TILES * N_PAIRS * 2 * 128], mybir.dt.float8e4,
        kind="ExternalInput"
    )
    out = nc.dram_tensor(
        "out", [B_PER_CORE, COUT, H * W_DIM], mybir.dt.float32, kind="ExternalOutput"
    )

    with tile.TileContext(nc) as tc:
        with (
            tc.tile_pool(name="wpool", bufs=1) as wpool,
            tc.tile_pool(name="xpool", bufs=1) as xpool,
            tc.tile_pool(name="opool", bufs=6) as opool,
            tc.tile_pool(name="pspool", bufs=7, space="PSUM") as pspool,
            tc.tile_pool(name="warmpool", bufs=1, space="PSUM") as warmpool,
        ):
            # Warm the PE clock (p-state ramp) while the input DMAs are in
            # flight: dependency-free matmuls on a memset scratch tile.
            scratch = opool.tile([128, 64], mybir.dt.bfloat16, name="warm_src", tag="wsrc")
            nc.vector.memset(scratch, 0.0)
            warm_ps = warmpool.tile([64, 64], mybir.dt.float32, name="warm_ps", tag="wps")
            for _ in range(64):
                nc.tensor.matmul(warm_ps, scratch[:, :64], scratch, start=True, stop=True)

            WCOLS = COUT_TILES * N_PAIRS * 2 * 128
            w_sb = wpool.tile([CIN, WCOLS], mybir.dt.float8e4, name="w_sb")
            WSPLITS = tuple(WCOLS * i // wchunks for i in range(wchunks)) + (WCOLS,)
            for lo, hi in zip(WSPLITS[:-1], WSPLITS[1:]):
                nc.scalar.dma_start(w_sb[:, lo:hi], wt[:, lo:hi])
            # [ci, ch, pair, two, co]
            w_view = w_sb.rearrange(
                "p (ch pair two co) -> p ch pair two co",
                ch=COUT_TILES, pair=N_PAIRS, two=2, co=128,
            )

            x_tiles = []
            for b in range(B_PER_CORE):
                xb = xpool.tile(
                    [CIN, 2, HP * WP], mybir.dt.float8e4, name=f"x_sb{b}", tag=f"x{b}"
                )
                # Only image 0 races the PE; later images load as one DMA.
                splits = tuple(xsplits) if b == 0 else (0, HP)
                for lo, hi in zip(splits[:-1], splits[1:]):
                    nc.sync.dma_start(
                        xb[:, :, lo * WP : hi * WP], xp[b, :, :, lo * WP : hi * WP]
                    )
                x_tiles.append(xb)

            def hl_window(xb, k, r0):
                # [128, 2(hi/lo), ROWS, 56] window for tap k at row group r0
                v = xb.rearrange("p two (h w) -> p two h w", w=WP)
                kh, kw = divmod(k, KW)
                return v[:, :, r0 + kh : r0 + kh + ROWS, kw : kw + W_DIM]

            def hi_pair_window(xb, ka, kb, r0):
                # [128, 2(tap a/b), ROWS, 56] overlapping window in the hi
                # plane; hand-built AP since the tap windows overlap.
                d = TAP_OFF[kb] - TAP_OFF[ka]
                off = xb.offset + r0 * WP + TAP_OFF[ka]
                return AP(
                    xb.tensor, off,
                    [[2 * HP * WP, CIN], [d, 2], [WP, ROWS], [1, W_DIM]],
                )

            for _rep in range(reps):
              for b in range(B_PER_CORE):
                for g in range(NG):
                    for c in range(COUT_TILES):
                        r0 = g * ROWS
                        ps = pspool.tile(
                            [128, NFREE], mybir.dt.float32, name="ps", tag="ps"
                        )
                        rhss = []
                        for k in range(KPOS):
                            rhss.append(hl_window(x_tiles[b], k, r0))
                        for ka, kb in HI_PAIRS:
                            rhss.append(hi_pair_window(x_tiles[b], ka, kb, r0))
                        rhss.append(hl_window(x_tiles[b], KPOS - 1, r0))
                        for p, rhs in enumerate(rhss):
                            nc.tensor.matmul(
                                ps, w_view[:, c, p], rhs,
                                start=(p == 0), stop=(p == N_PAIRS - 1),
                                perf_mode=mybir.MatmulPerfMode.DoubleRow,
                            )
                        ob = opool.tile(
                            [128, NFREE], mybir.dt.float32, name="ob", tag="ob"
                        )
                        nc.vector.tensor_scalar_mul(ob, ps, 1.0 / W_SCALE)
                        nc.sync.dma_start(
                            out[
                                b,
                                c * 128 : (c + 1) * 128,
                                r0 * W_DIM : (r0 + ROWS) * W_DIM,
                            ],
                            ob,
                        )
    nc.compile()
    return nc


def _get_nc() -> bass.Bass:
    global _NC_CACHE
    if _NC_CACHE is None:
        _NC_CACHE = build_nc()
    return _NC_CACHE


def _prep_inputs(x: np.ndarray, W: np.ndarray):
    x = np.asarray(x, dtype=np.float32)
    W = np.asarray(W, dtype=np.float32)
    f8 = ml_dtypes.float8_e4m3

    x_hi = x.astype(f8)
    x_lo = (x - x_hi.astype(np.float32)).astype(f8)
    xp = np.zeros((B, CIN, 2, HP, WP), dtype=f8)
    xp[:, :, 0, 1 : 1 + H, 1 : 1 + W_DIM] = x_hi
    xp[:, :, 1, 1 : 1 + H, 1 : 1 + W_DIM] = x_lo
    xp = xp.reshape(B, CIN, 2, HP * WP)

    Ws = W * W_SCALE
    W_hi = Ws.astype(f8)
    W_lo = (Ws - W_hi.astype(np.float32)).astype(f8)
    # [co, ci, kh, kw] -> [ci, k, co] per half
    def taps(Wq):
        return (
            Wq.astype(np.float32)
            .transpose(1, 2, 3, 0)            # [ci, kh, kw, co]
            .reshape(CIN, KPOS, COUT)
        )
    hi_t, lo_t = taps(W_hi), taps(W_lo)

    wt = np.zeros((CIN, COUT_TILES, N_PAIRS, 2, 128), dtype=np.float32)
    for ch in range(COUT_TILES):
        co = slice(ch * 128, (ch + 1) * 128)
        for k in range(KPOS):
            wt[:, ch, k, 0] = hi_t[:, k, co]
            wt[:, ch, k, 1] = hi_t[:, k, co]
        for j, (ka, kb) in enumerate(HI_PAIRS):
            wt[:, ch, KPOS + j, 0] = lo_t[:, ka, co]
            wt[:, ch, KPOS + j, 1] = lo_t[:, kb, co]
        wt[:, ch, N_PAIRS - 1, 0] = lo_t[:, KPOS - 1, co]
        wt[:, ch, N_PAIRS - 1, 1] = lo_t[:, KPOS - 1, co]
    wt = wt.reshape(CIN, -1).astype(f8)

    in_maps = []
    for c in range(N_CORES):
        in_maps.append(
            {
                "xp": np.ascontiguousarray(xp[c * B_PER_CORE : (c + 1) * B_PER_CORE]),
                "wt": wt,
            }
        )
    return in_maps


def kernel_run(x: np.ndarray, W: np.ndarray, **spmd_kwargs):
    """Run the conv and return (output, BassKernelResults)."""
    in_maps = _prep_inputs(x, W)
    res = run_bass_kernel_spmd(
        _get_nc(), in_maps, core_ids=list(range(N_CORES)), **spmd_kwargs
    )
    out = np.concatenate(
        [
            np.asarray(res.results[c]["out"], dtype=np.float32).reshape(
                B_PER_CORE, COUT, H, W_DIM
            )
            for c in range(N_CORES)
        ],
        axis=0,
    )
    return out, res


def kernel(x: np.ndarray, W: np.ndarray) -> np.ndarray:
    out, _ = kernel_run(x, W)
    return out
